# revision 13
# baseline (speedup 1.0000x reference)
"""Trainium2 Bass kernel for EnhancedOFTQKVLayer.

Computes out[b,s,o] = x[b,s,:] @ filt[o,:]^T + bias[o] where
filt = [Wq @ BD(cayley(q_R)); Wk @ BD(cayley(k_R)); Wv @ BD(cayley(v_R))]
(BD = block-diagonal, cayley(A) = (I-S) inv(I+S+eps I), S = 0.5(A-A^T)).

Distribution: data-parallel — batch b (8 rows) sharded one per NeuronCore;
attn_weight / bias / rotation blocks replicated. Each core:
  1. Cayley via SPD Newton-Schulz on P = (1+eps)^2 I - S^2 (all iterates are
     polynomials in S^2, hence symmetric -> lhsT=operand works without
     transposes; periodic symmetrization kills roundoff-asymmetry growth).
     fp16 iterations + fp32 polish. Blocks are processed in sets of 4 whose
     matmuls share one PSUM bank, so every elementwise step is one wide DVE
     op over [128, 512] instead of four narrow ones; emission is
     iteration-major so the independent sets pipeline densely on the PE.
  2. filtT[h,o] built on-chip: per 128-block, Q_n^T @ W_n^T with W^T obtained
     by PE-transpose of weight tiles. Stored bf16 in 48 (k, o-group) chunks.
  3. Main matmul in bf16 (fp32 PSUM accumulation): x row-tiles PE-transposed
     so the contraction dim sits on partitions; bias fused into the PSUM
     eviction on the vector engine.
"""

import numpy as np

import concourse.bass as bass
import concourse.mybir as mybir
import concourse.tile as tile
from concourse import bacc
from concourse.bass import ts
from concourse.masks import make_identity
from concourse.bass_utils import run_bass_kernel_spmd

F32 = mybir.dt.float32
F16 = mybir.dt.float16
BF16 = mybir.dt.bfloat16

MAIN_DT = BF16           # dtype of the big matmul inputs (x, filtT)

HIDDEN = 1024
OUT_DIM = 3 * HIDDEN
SEQ = 4096
P = 128
NBLK = 8                 # 128-blocks per hidden
NROT = 24                # 3 * NBLK rotation blocks
EPS = 1e-6
N_CORES = 8

NSETS = 6                # Newton processes blocks in sets of 4
SETB = 4

# Newton-Schulz schedule (validated offline against the jax reference).
NEWTON_F16 = 8
NEWTON_F32 = 1
SYM_ITERS = {3, 5, 7}    # symmetrize on these fp16 iterations
X0_A = 0.0152174         # X0 = aI + bP (degree-1 minimax init on [1, 260])
X0_B = -5.78922e-05
C0 = 2.0 / 261.0         # 2/(lam_min + lam_max_bound), lam_max(P) ~249 < 260

M_TILES = SEQ // P       # 32
O_TILES = OUT_DIM // 512  # 6


def build_body(ctx, tc):
    nc = tc.nc

    x = nc.dram_tensor("x", [SEQ, HIDDEN], F32, kind="ExternalInput").ap()
    w = nc.dram_tensor("w", [OUT_DIM, HIDDEN], F32, kind="ExternalInput").ap()
    bias = nc.dram_tensor("bias", [OUT_DIM], F32, kind="ExternalInput").ap()
    rmat = nc.dram_tensor("rmat", [NROT, P, P], F32, kind="ExternalInput").ap()
    out = nc.dram_tensor("out", [SEQ, OUT_DIM], F32, kind="ExternalOutput").ap()

    sub = nc.vector.tensor_sub
    add = nc.vector.tensor_add
    smul = nc.vector.tensor_scalar_mul
    cp = nc.vector.tensor_copy
    acp = nc.any.tensor_copy

    def bc(t):  # broadcast a [P, P] constant over a set's middle dim
        return t[:].unsqueeze(1).to_broadcast([P, SETB, P])

    # ---- persistent pools ----
    const = ctx.enter_context(tc.tile_pool(name="const", bufs=1))
    ftp = ctx.enter_context(tc.tile_pool(name="ftp", bufs=1))
    xp = ctx.enter_context(tc.tile_pool(name="xp", bufs=3))
    obp = ctx.enter_context(tc.tile_pool(name="obp", bufs=2))
    ps_g = ctx.enter_context(tc.tile_pool(name="ps_g", bufs=4, space="PSUM"))
    ps_xt = ctx.enter_context(tc.tile_pool(name="ps_xt", bufs=1, space="PSUM"))
    ps_po = ctx.enter_context(tc.tile_pool(name="ps_po", bufs=3, space="PSUM"))

    ident32 = const.tile([P, P], F32)
    make_identity(nc, ident32)
    identb = const.tile([P, P], MAIN_DT)
    cp(identb[:], ident32[:])
    eI2 = const.tile([P, P], F32)       # (1+eps)^2 I
    smul(eI2[:], ident32[:], float((1.0 + EPS) ** 2))
    eI12 = const.tile([P, P], F32)      # ((1+eps) + (1+eps)^2) I
    smul(eI12[:], ident32[:], float((1.0 + EPS) + (1.0 + EPS) ** 2))
    twoI = const.tile([P, P], F32)      # 2 I
    smul(twoI[:], ident32[:], 2.0)
    aI0 = const.tile([P, P], F16)       # X0_A * I  (Newton init)
    smul(aI0[:], ident32[:], float(X0_A))
    two_eye16 = const.tile([P, P], F16)  # 2 I (fp16, Newton rhs)
    smul(two_eye16[:], ident32[:], 2.0)

    bias_bc = const.tile([P, OUT_DIM], MAIN_DT)
    with tc.tile_pool(name="biasld", bufs=1) as bl:
        brow = bl.tile([1, OUT_DIM], F32)
        nc.sync.dma_start(brow[:], bias.unsqueeze(0))
        cp(bias_bc[:1, :], brow[:])
    nc.gpsimd.partition_broadcast(bias_bc[:], bias_bc[:1, :])

    # filtT chunks: ft[k][og][c, o'] = filtT[k*128+c, og*512+o']
    ft = [[ftp.tile([P, 512], MAIN_DT, tag=f"ft{k}_{og}", name=f"ft{k}_{og}")
           for og in range(O_TILES)] for k in range(NBLK)]

    # ---- phase A+B scoped pools ----
    with (
        tc.tile_pool(name="nper", bufs=1) as nper,     # per-set persistents
        tc.tile_pool(name="nx", bufs=1) as nxp,        # per-set X iterates
        tc.tile_pool(name="nrot", bufs=2) as nrot,     # rotating temps
        tc.tile_pool(name="qpool", bufs=1) as qpool,
        tc.tile_pool(name="wpool", bufs=2) as wpool,
        tc.tile_pool(name="wtsp", bufs=2) as wtsp,
    ):
        # x-tile prep shared by prefetch (below) and the main loop
        def emit_xprep(mt):
            xr = xp.tile([P, HIDDEN], F32, tag="xr", name=f"xr{mt}")
            nc.sync.dma_start(xr[:], x[ts(mt, P), :])
            xr16 = xp.tile([P, HIDDEN], MAIN_DT, tag="xr16", name=f"xr16_{mt}")
            acp(xr16[:], xr[:])
            xt = xp.tile([P, NBLK, P], MAIN_DT, tag="xt", name=f"xt{mt}")
            tpg = ps_xt.tile([P, NBLK, P], MAIN_DT, tag="xtp",
                             name=f"xtp{mt}")
            for k in range(NBLK):
                nc.tensor.transpose(tpg[:, k, :], xr16[:, ts(k, P)],
                                    identb[:])
            acp(xt[:], tpg[:])
            return xt
        # ---------- Phase A: Newton-Cayley, 6 sets of 4 blocks ----------
        s_s, p32_s, p16_s, x_s = [], [], [], []
        for s in range(NSETS):
            n0 = s * SETB
            aset = nrot.tile([P, SETB, P], F32, tag="a")
            nc.sync.dma_start(aset[:],
                              rmat[n0:n0 + SETB].rearrange("n p f -> p n f"))
            tpg = ps_g.tile([P, SETB, P], F32, tag="g")
            for j in range(SETB):
                nc.tensor.transpose(tpg[:, j, :], aset[:, j, :], ident32[:])
            sset = nper.tile([P, SETB, P], F32, tag=f"s{s}", name=f"s{s}")
            sub(sset[:], aset[:], tpg[:])
            smul(sset[:], sset[:], 0.5)                  # S
            g = ps_g.tile([P, SETB, P], F32, tag="g")
            for j in range(SETB):                        # S^T @ S = -S^2
                nc.tensor.matmul(g[:, j, :], lhsT=sset[:, j, :],
                                 rhs=sset[:, j, :], start=True, stop=True)
            p32s = nper.tile([P, SETB, P], F32, tag=f"p32{s}", name=f"p32{s}")
            add(p32s[:], bc(eI2), g[:])                  # P = (1+e)^2 I - S^2
            p16s = nper.tile([P, SETB, P], F16, tag=f"p16{s}", name=f"p16{s}")
            acp(p16s[:], p32s[:])
            xset = nxp.tile([P, SETB, P], F16, tag=f"x{s}", name=f"x{s}_init")
            smul(xset[:], p32s[:], float(X0_B))          # X0 = aI + bP
            add(xset[:], xset[:], bc(aI0))
            s_s.append(sset)
            p32_s.append(p32s)
            p16_s.append(p16s)
            x_s.append(xset)

        PREFETCH = 8
        xt_pre = [emit_xprep(mt) for mt in range(PREFETCH)]

        for i in range(NEWTON_F16):
            do_sym = i in SYM_ITERS
            for s in range(NSETS):
                g1 = ps_g.tile([P, SETB, P], F32, tag="g")
                for j in range(SETB):
                    nc.tensor.matmul(g1[:, j, :], lhsT=p16_s[s][:, j, :],
                                     rhs=x_s[s][:, j, :], start=True, stop=True)
                t1n = nrot.tile([P, SETB, P], F16, tag="t1n")
                smul(t1n[:], g1[:], -1.0)                # -T1 (DVE: fast sems)
                g2 = ps_g.tile([P, SETB, P], F32, tag="g")
                for j in range(SETB):                    # X' = X(2I) - X T1
                    nc.tensor.matmul(g2[:, j, :], lhsT=x_s[s][:, j, :],
                                     rhs=two_eye16[:], start=True, stop=False)
                    nc.tensor.matmul(g2[:, j, :], lhsT=x_s[s][:, j, :],
                                     rhs=t1n[:, j, :], start=False, stop=True)
                xset = nxp.tile([P, SETB, P], F16, tag=f"x{s}",
                                name=f"x{s}_{i}")
                if not do_sym:
                    acp(xset[:], g2[:])
                else:
                    xc = nrot.tile([P, SETB, P], F32, tag="xc")
                    cp(xc[:], g2[:])
                    tpg = ps_g.tile([P, SETB, P], F32, tag="g")
                    for j in range(SETB):
                        nc.tensor.transpose(tpg[:, j, :], xc[:, j, :],
                                            ident32[:])
                    add(xc[:], xc[:], tpg[:])
                    nc.scalar.activation(xset[:], xc[:],
                                         mybir.ActivationFunctionType.Copy,
                                         scale=0.5)
                x_s[s] = xset

        xf_s = []
        for s in range(NSETS):
            xf = nxp.tile([P, SETB, P], F32, tag=f"xf{s}", name=f"xf{s}_init")
            acp(xf[:], x_s[s][:])
            xf_s.append(xf)
        for i in range(NEWTON_F32):
            for s in range(NSETS):
                g1 = ps_g.tile([P, SETB, P], F32, tag="g")
                for j in range(SETB):
                    nc.tensor.matmul(g1[:, j, :], lhsT=p32_s[s][:, j, :],
                                     rhs=xf_s[s][:, j, :], start=True,
                                     stop=True)
                uf = nrot.tile([P, SETB, P], F32, tag="uf")
                sub(uf[:], bc(twoI), g1[:])
                g2 = ps_g.tile([P, SETB, P], F32, tag="g")
                for j in range(SETB):
                    nc.tensor.matmul(g2[:, j, :], lhsT=xf_s[s][:, j, :],
                                     rhs=uf[:, j, :], start=True, stop=True)
                xf = nxp.tile([P, SETB, P], F32, tag=f"xf{s}",
                              name=f"xf{s}_{i}")
                acp(xf[:], g2[:])
                xf_s[s] = xf

        # Q = B @ X with B^T = (1+e)I + (2+e)S + S^2 = eI12 + (2+e)S - P
        q_s = []
        for s in range(NSETS):
            bt = nrot.tile([P, SETB, P], F32, tag="bt")
            nc.vector.tensor_scalar(bt[:], s_s[s][:], float(2.0 + EPS), None,
                                    mybir.AluOpType.mult)
            add(bt[:], bt[:], bc(eI12))
            sub(bt[:], bt[:], p32_s[s][:])
            g = ps_g.tile([P, SETB, P], F32, tag="g")
            for j in range(SETB):
                nc.tensor.matmul(g[:, j, :], lhsT=bt[:, j, :],
                                 rhs=xf_s[s][:, j, :], start=True, stop=True)
            qset = qpool.tile([P, SETB, P], MAIN_DT, tag=f"q{s}", name=f"q{s}")
            acp(qset[:], g[:])
            q_s.append(qset)

        def q_lhsT(n):
            return q_s[n // SETB][:, n % SETB, :]

        # ---------- Phase B: W^T (streamed in 512-col groups) then filtT ----------
        for og in range(O_TILES):          # six 512-wide output column groups
            part = og // 2                 # which of q/k/v this group belongs to
            wts = wtsp.tile([P, NBLK, 512], MAIN_DT, tag="wts")
            for j4 in range(4):            # four 128-row W tiles per group
                ot = og * 4 + j4
                wrow = wpool.tile([P, HIDDEN], F32, tag="wrow")
                nc.sync.dma_start(wrow[:], w[ts(ot, P), :])
                for kh in range(2):                     # two 4-block groups
                    tpg = ps_g.tile([P, SETB, P], F32, tag="g")
                    for k4 in range(SETB):
                        k = kh * SETB + k4
                        nc.tensor.transpose(tpg[:, k4, :], wrow[:, ts(k, P)],
                                            ident32[:])
                    acp(wts[:, ts(kh, SETB), ts(j4, P)], tpg[:])
            for k in range(NBLK):
                fg = ps_g.tile([P, 512], F32, tag="g")
                nc.tensor.matmul(fg[:], lhsT=q_lhsT(part * NBLK + k),
                                 rhs=wts[:, k, :], start=True, stop=True)
                acp(ft[k][og][:], fg[:])

        _CACHE["xt_pre"] = xt_pre

        # ---------- Phase C: main matmul (o-outer so each (mt, o) group ----------
        # only depends on its own filtT column chunk, overlapping with B)
        xt_pre = _CACHE.pop("xt_pre")
        for mt in range(M_TILES):
            xt = xt_pre[mt] if mt < len(xt_pre) else emit_xprep(mt)
            for o in range(O_TILES):
                pg = ps_po.tile([P, 512], F32, tag="po", name=f"po{mt}_{o}")
                for k in range(NBLK):
                    nc.tensor.matmul(pg[:], lhsT=xt[:, k, :], rhs=ft[k][o][:],
                                     start=(k == 0), stop=(k == NBLK - 1))
                ob = obp.tile([P, 512], F32, tag="ob", name=f"ob{mt}_{o}")
                add(ob[:], pg[:], bias_bc[:, ts(o, 512)])
                nc.sync.dma_start(out[ts(mt, P), ts(o, 512)], ob[:])


_CACHE = {}


def build():
    if "nc" in _CACHE:
        return _CACHE["nc"]
    import contextlib

    nc = bacc.Bacc("TRN2", target_bir_lowering=False, debug=False)
    with tile.TileContext(nc) as tc:
        with contextlib.ExitStack() as ctx:
            build_body(ctx, tc)
    nc.compile()
    _CACHE["nc"] = nc
    return nc


def make_in_maps(attn_weight, bias, x, q_R, k_R, v_R):
    rmat = np.ascontiguousarray(
        np.concatenate([q_R, k_R, v_R], axis=0), dtype=np.float32)
    w = np.ascontiguousarray(attn_weight, dtype=np.float32)
    b = np.ascontiguousarray(bias, dtype=np.float32)
    return [
        {"x": np.ascontiguousarray(x[c], dtype=np.float32),
         "w": w, "bias": b, "rmat": rmat}
        for c in range(N_CORES)
    ]


def kernel(attn_weight, bias, x, q_R, k_R, v_R, **run_kwargs):
    nc = build()
    in_maps = make_in_maps(attn_weight, bias, x, q_R, k_R, v_R)
    res = run_bass_kernel_spmd(nc, in_maps, core_ids=list(range(N_CORES)),
                               **run_kwargs)
    out = np.stack([res.results[c]["out"] for c in range(N_CORES)], axis=0)
    _CACHE["last_results"] = res
    return out


# revision 14
# speedup vs baseline: 1.0702x; 1.0702x over previous
"""Trainium2 Bass kernel for EnhancedOFTQKVLayer.

Computes out[b,s,o] = x[b,s,:] @ filt[o,:]^T + bias[o] where
filt = [Wq @ BD(cayley(q_R)); Wk @ BD(cayley(k_R)); Wv @ BD(cayley(v_R))]
(BD = block-diagonal, cayley(A) = (I-S) inv(I+S+eps I), S = 0.5(A-A^T)).

Distribution: data-parallel — batch b (8 rows) sharded one per NeuronCore;
attn_weight / bias / rotation blocks replicated. Each core:
  1. Cayley via SPD Newton-Schulz on P = (1+eps)^2 I - S^2 (all iterates are
     polynomials in S^2, hence symmetric -> lhsT=operand works without
     transposes; periodic symmetrization kills roundoff-asymmetry growth).
     fp16 iterations + fp32 polish. Blocks are processed in sets of 4 whose
     matmuls share one PSUM bank, so every elementwise step is one wide DVE
     op over [128, 512] instead of four narrow ones; emission is
     iteration-major so the independent sets pipeline densely on the PE.
  2. filtT[h,o] built on-chip: per 128-block, Q_n^T @ W_n^T with W^T obtained
     by PE-transpose of weight tiles. Stored bf16 in 48 (k, o-group) chunks.
  3. Main matmul in bf16 (fp32 PSUM accumulation): x row-tiles PE-transposed
     so the contraction dim sits on partitions; bias fused into the PSUM
     eviction on the vector engine.
"""

import numpy as np

import concourse.bass as bass
import concourse.mybir as mybir
import concourse.tile as tile
from concourse import bacc
from concourse.bass import ts
from concourse.masks import make_identity
from concourse.bass_utils import run_bass_kernel_spmd

F32 = mybir.dt.float32
F16 = mybir.dt.float16
BF16 = mybir.dt.bfloat16

MAIN_DT = BF16           # dtype of the big matmul inputs (x, filtT)

HIDDEN = 1024
OUT_DIM = 3 * HIDDEN
SEQ = 4096
P = 128
NBLK = 8                 # 128-blocks per hidden
NROT = 24                # 3 * NBLK rotation blocks
EPS = 1e-6
N_CORES = 8

NSETS = 6                # Newton processes blocks in sets of 4
SETB = 4

# Newton-Schulz schedule (validated offline against the jax reference).
NEWTON_F16 = 8
NEWTON_F32 = 1
SYM_ITERS = {3, 5, 7}    # symmetrize on these fp16 iterations
X0_A = 0.0152174         # X0 = aI + bP (degree-1 minimax init on [1, 260])
X0_B = -5.78922e-05
C0 = 2.0 / 261.0         # 2/(lam_min + lam_max_bound), lam_max(P) ~249 < 260

M_TILES = SEQ // P       # 32
O_TILES = OUT_DIM // 512  # 6


def build_body(ctx, tc):
    nc = tc.nc

    x = nc.dram_tensor("x", [SEQ, HIDDEN], F32, kind="ExternalInput").ap()
    w = nc.dram_tensor("w", [OUT_DIM, HIDDEN], F32, kind="ExternalInput").ap()
    bias = nc.dram_tensor("bias", [OUT_DIM], F32, kind="ExternalInput").ap()
    rmat = nc.dram_tensor("rmat", [NROT, P, P], F32, kind="ExternalInput").ap()
    out = nc.dram_tensor("out", [SEQ, OUT_DIM], F32, kind="ExternalOutput").ap()

    sub = nc.vector.tensor_sub
    add = nc.vector.tensor_add
    smul = nc.vector.tensor_scalar_mul
    cp = nc.vector.tensor_copy
    acp = nc.any.tensor_copy

    def bc(t):  # broadcast a [P, P] constant over a set's middle dim
        return t[:].unsqueeze(1).to_broadcast([P, SETB, P])

    # ---- persistent pools ----
    const = ctx.enter_context(tc.tile_pool(name="const", bufs=1))
    ftp = ctx.enter_context(tc.tile_pool(name="ftp", bufs=1))
    xp = ctx.enter_context(tc.tile_pool(name="xp", bufs=3))
    obp = ctx.enter_context(tc.tile_pool(name="obp", bufs=2))
    ps_xt = ctx.enter_context(tc.tile_pool(name="ps_xt", bufs=2, space="PSUM"))

    ident32 = const.tile([P, P], F32)
    make_identity(nc, ident32)
    identb = const.tile([P, P], MAIN_DT)
    cp(identb[:], ident32[:])
    eI2 = const.tile([P, P], F32)       # (1+eps)^2 I
    smul(eI2[:], ident32[:], float((1.0 + EPS) ** 2))
    eI12 = const.tile([P, P], F32)      # ((1+eps) + (1+eps)^2) I
    smul(eI12[:], ident32[:], float((1.0 + EPS) + (1.0 + EPS) ** 2))
    twoI = const.tile([P, P], F32)      # 2 I
    smul(twoI[:], ident32[:], 2.0)
    aI0 = const.tile([P, P], F16)       # X0_A * I  (Newton init)
    smul(aI0[:], ident32[:], float(X0_A))
    two_eye16 = const.tile([P, P], F16)  # 2 I (fp16, Newton rhs)
    smul(two_eye16[:], ident32[:], 2.0)

    bias_bc = const.tile([P, OUT_DIM], MAIN_DT)
    with tc.tile_pool(name="biasld", bufs=1) as bl:
        brow = bl.tile([1, OUT_DIM], F32)
        nc.sync.dma_start(brow[:], bias.unsqueeze(0))
        cp(bias_bc[:1, :], brow[:])
    nc.gpsimd.partition_broadcast(bias_bc[:], bias_bc[:1, :])

    # filtT chunks: ft[k][og][c, o'] = filtT[k*128+c, og*512+o']
    ft = [[ftp.tile([P, 512], MAIN_DT, tag=f"ft{k}_{og}", name=f"ft{k}_{og}")
           for og in range(O_TILES)] for k in range(NBLK)]

    # ---- phase A+B scoped pools ----
    with (
        tc.tile_pool(name="nper", bufs=1) as nper,     # per-set persistents
        tc.tile_pool(name="nx", bufs=1) as nxp,        # per-set X iterates
        tc.tile_pool(name="nrot", bufs=2) as nrot,     # rotating temps
        tc.tile_pool(name="qpool", bufs=1) as qpool,
        tc.tile_pool(name="wpool", bufs=2) as wpool,
        tc.tile_pool(name="wtsp", bufs=2) as wtsp,
        tc.tile_pool(name="ps_g", bufs=4, space="PSUM") as ps_g,
        tc.tile_pool(name="ps_tp", bufs=2, space="PSUM") as ps_tp,
    ):
        # x-tile prep shared by prefetch (below) and the main loop
        def emit_xprep(mt):
            xr = xp.tile([P, HIDDEN], F32, tag="xr", name=f"xr{mt}")
            nc.sync.dma_start(xr[:], x[ts(mt, P), :])
            xr16 = xp.tile([P, HIDDEN], MAIN_DT, tag="xr16", name=f"xr16_{mt}")
            acp(xr16[:], xr[:])
            xt = xp.tile([P, NBLK, P], MAIN_DT, tag="xt", name=f"xt{mt}")
            tpg = ps_xt.tile([P, NBLK, P], MAIN_DT, tag="xtp",
                             name=f"xtp{mt}")
            for k in range(NBLK):
                nc.tensor.transpose(tpg[:, k, :], xr16[:, ts(k, P)],
                                    identb[:])
            acp(xt[:], tpg[:])
            return xt
        # ---------- Phase A: Newton-Cayley, 6 sets of 4 blocks ----------
        s_s, p32_s, p16_s, x_s = [], [], [], []
        for s in range(NSETS):
            n0 = s * SETB
            aset = nrot.tile([P, SETB, P], F32, tag="a")
            nc.sync.dma_start(aset[:],
                              rmat[n0:n0 + SETB].rearrange("n p f -> p n f"))
            tpg = ps_tp.tile([P, SETB, P], F32, tag="tp")
            for j in range(SETB):
                nc.tensor.transpose(tpg[:, j, :], aset[:, j, :], ident32[:])
            sset = nper.tile([P, SETB, P], F32, tag=f"s{s}", name=f"s{s}")
            sub(sset[:], aset[:], tpg[:])
            smul(sset[:], sset[:], 0.5)                  # S
            g = ps_g.tile([P, SETB, P], F32, tag="g")
            for j in range(SETB):                        # S^T @ S = -S^2
                nc.tensor.matmul(g[:, j, :], lhsT=sset[:, j, :],
                                 rhs=sset[:, j, :], start=True, stop=True)
            p32s = nper.tile([P, SETB, P], F32, tag=f"p32{s}", name=f"p32{s}")
            add(p32s[:], bc(eI2), g[:])                  # P = (1+e)^2 I - S^2
            p16s = nper.tile([P, SETB, P], F16, tag=f"p16{s}", name=f"p16{s}")
            acp(p16s[:], p32s[:])
            xset = nxp.tile([P, SETB, P], F16, tag=f"x{s}", name=f"x{s}_init")
            smul(xset[:], p32s[:], float(X0_B))          # X0 = aI + bP
            add(xset[:], xset[:], bc(aI0))
            s_s.append(sset)
            p32_s.append(p32s)
            p16_s.append(p16s)
            x_s.append(xset)

        PREFETCH = 8
        xt_pre = [emit_xprep(mt) for mt in range(PREFETCH)]

        for i in range(NEWTON_F16):
            do_sym = i in SYM_ITERS
            for s in range(NSETS):
                g1 = ps_g.tile([P, SETB, P], F32, tag="g")
                for j in range(SETB):
                    nc.tensor.matmul(g1[:, j, :], lhsT=p16_s[s][:, j, :],
                                     rhs=x_s[s][:, j, :], start=True, stop=True)
                t1n = nrot.tile([P, SETB, P], F16, tag="t1n")
                nc.scalar.activation(t1n[:], g1[:],      # -T1, off the DVE
                                     mybir.ActivationFunctionType.Copy,
                                     scale=-1.0)
                g2 = ps_g.tile([P, SETB, P], F32, tag="g")
                for j in range(SETB):                    # X' = X(2I) - X T1
                    nc.tensor.matmul(g2[:, j, :], lhsT=x_s[s][:, j, :],
                                     rhs=two_eye16[:], start=True, stop=False)
                    nc.tensor.matmul(g2[:, j, :], lhsT=x_s[s][:, j, :],
                                     rhs=t1n[:, j, :], start=False, stop=True)
                xset = nxp.tile([P, SETB, P], F16, tag=f"x{s}",
                                name=f"x{s}_{i}")
                if not do_sym:
                    acp(xset[:], g2[:])
                else:
                    xc = nrot.tile([P, SETB, P], F32, tag="xc")
                    cp(xc[:], g2[:])
                    tpg = ps_tp.tile([P, SETB, P], F32, tag="tp")
                    for j in range(SETB):
                        nc.tensor.transpose(tpg[:, j, :], xc[:, j, :],
                                            ident32[:])
                    add(xc[:], xc[:], tpg[:])
                    nc.scalar.activation(xset[:], xc[:],
                                         mybir.ActivationFunctionType.Copy,
                                         scale=0.5)
                x_s[s] = xset

        xf_s = []
        for s in range(NSETS):
            xf = nxp.tile([P, SETB, P], F32, tag=f"xf{s}", name=f"xf{s}_init")
            acp(xf[:], x_s[s][:])
            xf_s.append(xf)
        for i in range(NEWTON_F32):
            for s in range(NSETS):
                g1 = ps_g.tile([P, SETB, P], F32, tag="g")
                for j in range(SETB):
                    nc.tensor.matmul(g1[:, j, :], lhsT=p32_s[s][:, j, :],
                                     rhs=xf_s[s][:, j, :], start=True,
                                     stop=True)
                uf = nrot.tile([P, SETB, P], F32, tag="uf")
                sub(uf[:], bc(twoI), g1[:])
                g2 = ps_g.tile([P, SETB, P], F32, tag="g")
                for j in range(SETB):
                    nc.tensor.matmul(g2[:, j, :], lhsT=xf_s[s][:, j, :],
                                     rhs=uf[:, j, :], start=True, stop=True)
                xf = nxp.tile([P, SETB, P], F32, tag=f"xf{s}",
                              name=f"xf{s}_{i}")
                acp(xf[:], g2[:])
                xf_s[s] = xf

        # Q = B @ X with B^T = (1+e)I + (2+e)S + S^2 = eI12 + (2+e)S - P
        q_s = []
        for s in range(NSETS):
            bt = nrot.tile([P, SETB, P], F32, tag="bt")
            nc.vector.tensor_scalar(bt[:], s_s[s][:], float(2.0 + EPS), None,
                                    mybir.AluOpType.mult)
            add(bt[:], bt[:], bc(eI12))
            sub(bt[:], bt[:], p32_s[s][:])
            g = ps_g.tile([P, SETB, P], F32, tag="g")
            for j in range(SETB):
                nc.tensor.matmul(g[:, j, :], lhsT=bt[:, j, :],
                                 rhs=xf_s[s][:, j, :], start=True, stop=True)
            qset = qpool.tile([P, SETB, P], MAIN_DT, tag=f"q{s}", name=f"q{s}")
            acp(qset[:], g[:])
            q_s.append(qset)

        def q_lhsT(n):
            return q_s[n // SETB][:, n % SETB, :]

        # ---------- Phase B: W^T (streamed in 512-col groups) then filtT ----------
        for og in range(O_TILES):          # six 512-wide output column groups
            part = og // 2                 # which of q/k/v this group belongs to
            wts = wtsp.tile([P, NBLK, 512], MAIN_DT, tag="wts")
            for j4 in range(4):            # four 128-row W tiles per group
                ot = og * 4 + j4
                wrow = wpool.tile([P, HIDDEN], F32, tag="wrow")
                nc.sync.dma_start(wrow[:], w[ts(ot, P), :])
                for kh in range(2):                     # two 4-block groups
                    tpg = ps_tp.tile([P, SETB, P], F32, tag="tp")
                    for k4 in range(SETB):
                        k = kh * SETB + k4
                        nc.tensor.transpose(tpg[:, k4, :], wrow[:, ts(k, P)],
                                            ident32[:])
                    acp(wts[:, ts(kh, SETB), ts(j4, P)], tpg[:])
            for k in range(NBLK):
                fg = ps_g.tile([P, 512], F32, tag="g")
                nc.tensor.matmul(fg[:], lhsT=q_lhsT(part * NBLK + k),
                                 rhs=wts[:, k, :], start=True, stop=True)
                acp(ft[k][og][:], fg[:])

        _CACHE["xt_pre"] = xt_pre

    # ---------- Phase C: main matmul ----------
    xt_pre = _CACHE.pop("xt_pre")
    with tc.tile_pool(name="ps_out", bufs=6, space="PSUM") as ps_out:
        for mt in range(M_TILES):
            xt = xt_pre[mt] if mt < len(xt_pre) else emit_xprep(mt)
            psums = [ps_out.tile([P, 512], F32, tag="po", name=f"po{mt}_{i}")
                     for i in range(O_TILES)]
            for k in range(NBLK):
                for o in range(O_TILES):
                    nc.tensor.matmul(psums[o][:], lhsT=xt[:, k, :],
                                     rhs=ft[k][o][:],
                                     start=(k == 0), stop=(k == NBLK - 1))
            for o in range(O_TILES):
                ob = obp.tile([P, 512], F32, tag="ob", name=f"ob{mt}_{o}")
                add(ob[:], psums[o][:], bias_bc[:, ts(o, 512)])
                nc.sync.dma_start(out[ts(mt, P), ts(o, 512)], ob[:])


_CACHE = {}


def build():
    if "nc" in _CACHE:
        return _CACHE["nc"]
    import contextlib

    nc = bacc.Bacc("TRN2", target_bir_lowering=False, debug=False)
    with tile.TileContext(nc) as tc:
        with contextlib.ExitStack() as ctx:
            build_body(ctx, tc)
    nc.compile()
    _CACHE["nc"] = nc
    return nc


def make_in_maps(attn_weight, bias, x, q_R, k_R, v_R):
    rmat = np.ascontiguousarray(
        np.concatenate([q_R, k_R, v_R], axis=0), dtype=np.float32)
    w = np.ascontiguousarray(attn_weight, dtype=np.float32)
    b = np.ascontiguousarray(bias, dtype=np.float32)
    return [
        {"x": np.ascontiguousarray(x[c], dtype=np.float32),
         "w": w, "bias": b, "rmat": rmat}
        for c in range(N_CORES)
    ]


def kernel(attn_weight, bias, x, q_R, k_R, v_R, **run_kwargs):
    nc = build()
    in_maps = make_in_maps(attn_weight, bias, x, q_R, k_R, v_R)
    res = run_bass_kernel_spmd(nc, in_maps, core_ids=list(range(N_CORES)),
                               **run_kwargs)
    out = np.stack([res.results[c]["out"] for c in range(N_CORES)], axis=0)
    _CACHE["last_results"] = res
    return out


# revision 15
# speedup vs baseline: 1.0784x; 1.0077x over previous
"""Trainium2 Bass kernel for EnhancedOFTQKVLayer.

Computes out[b,s,o] = x[b,s,:] @ filt[o,:]^T + bias[o] where
filt = [Wq @ BD(cayley(q_R)); Wk @ BD(cayley(k_R)); Wv @ BD(cayley(v_R))]
(BD = block-diagonal, cayley(A) = (I-S) inv(I+S+eps I), S = 0.5(A-A^T)).

Distribution: data-parallel — batch b (8 rows) sharded one per NeuronCore;
attn_weight / bias / rotation blocks replicated. Each core:
  1. Cayley via SPD Newton-Schulz on P = (1+eps)^2 I - S^2 (all iterates are
     polynomials in S^2, hence symmetric -> lhsT=operand works without
     transposes; periodic symmetrization kills roundoff-asymmetry growth).
     fp16 iterations + fp32 polish. Blocks are processed in sets of 4 whose
     matmuls share one PSUM bank, so every elementwise step is one wide DVE
     op over [128, 512] instead of four narrow ones; emission is
     iteration-major so the independent sets pipeline densely on the PE.
  2. filtT[h,o] built on-chip: per 128-block, Q_n^T @ W_n^T with W^T obtained
     by PE-transpose of weight tiles. Stored bf16 in 48 (k, o-group) chunks.
  3. Main matmul in bf16 (fp32 PSUM accumulation): x row-tiles PE-transposed
     so the contraction dim sits on partitions; bias fused into the PSUM
     eviction on the vector engine.
"""

import numpy as np

import concourse.bass as bass
import concourse.mybir as mybir
import concourse.tile as tile
from concourse import bacc
from concourse.bass import ts
from concourse.masks import make_identity
from concourse.bass_utils import run_bass_kernel_spmd

F32 = mybir.dt.float32
F16 = mybir.dt.float16
BF16 = mybir.dt.bfloat16

MAIN_DT = BF16           # dtype of the big matmul inputs (x, filtT)

HIDDEN = 1024
OUT_DIM = 3 * HIDDEN
SEQ = 4096
P = 128
NBLK = 8                 # 128-blocks per hidden
NROT = 24                # 3 * NBLK rotation blocks
EPS = 1e-6
N_CORES = 8

NSETS = 6                # Newton processes blocks in sets of 4
SETB = 4

# Newton-Schulz schedule (validated offline against the jax reference).
NEWTON_F16 = 8
NEWTON_F32 = 1
SYM_ITERS = {3, 5, 7}    # symmetrize on these fp16 iterations
X0_A = 0.0152174         # X0 = aI + bP (degree-1 minimax init on [1, 260])
X0_B = -5.78922e-05
C0 = 2.0 / 261.0         # 2/(lam_min + lam_max_bound), lam_max(P) ~249 < 260

M_TILES = SEQ // P       # 32
O_TILES = OUT_DIM // 512  # 6


def build_body(ctx, tc):
    nc = tc.nc

    x = nc.dram_tensor("x", [SEQ, HIDDEN], F32, kind="ExternalInput").ap()
    w = nc.dram_tensor("w", [OUT_DIM, HIDDEN], F32, kind="ExternalInput").ap()
    bias = nc.dram_tensor("bias", [OUT_DIM], F32, kind="ExternalInput").ap()
    rmat = nc.dram_tensor("rmat", [NROT, P, P], F32, kind="ExternalInput").ap()
    out = nc.dram_tensor("out", [SEQ, OUT_DIM], F32, kind="ExternalOutput").ap()

    sub = nc.vector.tensor_sub
    add = nc.vector.tensor_add
    smul = nc.vector.tensor_scalar_mul
    cp = nc.vector.tensor_copy
    acp = nc.any.tensor_copy

    def bc(t):  # broadcast a [P, P] constant over a set's middle dim
        return t[:].unsqueeze(1).to_broadcast([P, SETB, P])

    # ---- persistent pools ----
    const = ctx.enter_context(tc.tile_pool(name="const", bufs=1))
    ftp = ctx.enter_context(tc.tile_pool(name="ftp", bufs=1))
    xp = ctx.enter_context(tc.tile_pool(name="xp", bufs=3))
    obp = ctx.enter_context(tc.tile_pool(name="obp", bufs=2))
    ps_xt = ctx.enter_context(tc.tile_pool(name="ps_xt", bufs=2, space="PSUM"))

    ident32 = const.tile([P, P], F32)
    make_identity(nc, ident32)
    identb = const.tile([P, P], MAIN_DT)
    cp(identb[:], ident32[:])
    eI2 = const.tile([P, P], F32)       # (1+eps)^2 I
    smul(eI2[:], ident32[:], float((1.0 + EPS) ** 2))
    eI12 = const.tile([P, P], F32)      # ((1+eps) + (1+eps)^2) I
    smul(eI12[:], ident32[:], float((1.0 + EPS) + (1.0 + EPS) ** 2))
    twoI = const.tile([P, P], F32)      # 2 I
    smul(twoI[:], ident32[:], 2.0)
    aI0 = const.tile([P, P], F16)       # X0_A * I  (Newton init)
    smul(aI0[:], ident32[:], float(X0_A))
    two_eye16 = const.tile([P, P], F16)  # 2 I (fp16, Newton rhs)
    smul(two_eye16[:], ident32[:], 2.0)

    bias_bc = const.tile([P, OUT_DIM], MAIN_DT)
    with tc.tile_pool(name="biasld", bufs=1) as bl:
        brow = bl.tile([1, OUT_DIM], F32)
        nc.sync.dma_start(brow[:], bias.unsqueeze(0))
        cp(bias_bc[:1, :], brow[:])
    nc.gpsimd.partition_broadcast(bias_bc[:], bias_bc[:1, :])

    # filtT chunks: ft[k][og][c, o'] = filtT[k*128+c, og*512+o']
    ft = [[ftp.tile([P, 512], MAIN_DT, tag=f"ft{k}_{og}", name=f"ft{k}_{og}")
           for og in range(O_TILES)] for k in range(NBLK)]

    # ---- phase A+B scoped pools ----
    with (
        tc.tile_pool(name="nper", bufs=1) as nper,     # per-set persistents
        tc.tile_pool(name="nx", bufs=1) as nxp,        # per-set X iterates
        tc.tile_pool(name="nrot", bufs=2) as nrot,     # rotating temps
        tc.tile_pool(name="qpool", bufs=1) as qpool,
        tc.tile_pool(name="wpool", bufs=2) as wpool,
        tc.tile_pool(name="wtsp", bufs=2) as wtsp,
        tc.tile_pool(name="ps_g", bufs=4, space="PSUM") as ps_g,
        tc.tile_pool(name="ps_tp", bufs=2, space="PSUM") as ps_tp,
    ):
        # x-tile prep shared by prefetch (below) and the main loop
        def emit_xprep(mt):
            xr = xp.tile([P, HIDDEN], F32, tag="xr", name=f"xr{mt}")
            nc.sync.dma_start(xr[:], x[ts(mt, P), :])
            xr16 = xp.tile([P, HIDDEN], MAIN_DT, tag="xr16", name=f"xr16_{mt}")
            acp(xr16[:], xr[:])
            xt = xp.tile([P, NBLK, P], MAIN_DT, tag="xt", name=f"xt{mt}", bufs=10)
            tpg = ps_xt.tile([P, NBLK, P], MAIN_DT, tag="xtp",
                             name=f"xtp{mt}")
            for k in range(NBLK):
                nc.tensor.transpose(tpg[:, k, :], xr16[:, ts(k, P)],
                                    identb[:])
            acp(xt[:], tpg[:])
            return xt
        # ---------- Phase A: Newton-Cayley, 6 sets of 4 blocks ----------
        s_s, p32_s, p16_s, x_s = [], [], [], []
        for s in range(NSETS):
            n0 = s * SETB
            aset = nrot.tile([P, SETB, P], F32, tag="a")
            nc.sync.dma_start(aset[:],
                              rmat[n0:n0 + SETB].rearrange("n p f -> p n f"))
            tpg = ps_tp.tile([P, SETB, P], F32, tag="tp")
            for j in range(SETB):
                nc.tensor.transpose(tpg[:, j, :], aset[:, j, :], ident32[:])
            sset = nper.tile([P, SETB, P], F32, tag=f"s{s}", name=f"s{s}")
            sub(sset[:], aset[:], tpg[:])
            smul(sset[:], sset[:], 0.5)                  # S
            g = ps_g.tile([P, SETB, P], F32, tag="g")
            for j in range(SETB):                        # S^T @ S = -S^2
                nc.tensor.matmul(g[:, j, :], lhsT=sset[:, j, :],
                                 rhs=sset[:, j, :], start=True, stop=True)
            p32s = nper.tile([P, SETB, P], F32, tag=f"p32{s}", name=f"p32{s}")
            add(p32s[:], bc(eI2), g[:])                  # P = (1+e)^2 I - S^2
            p16s = nper.tile([P, SETB, P], F16, tag=f"p16{s}", name=f"p16{s}")
            acp(p16s[:], p32s[:])
            xset = nxp.tile([P, SETB, P], F16, tag=f"x{s}", name=f"x{s}_init")
            smul(xset[:], p32s[:], float(X0_B))          # X0 = aI + bP
            add(xset[:], xset[:], bc(aI0))
            s_s.append(sset)
            p32_s.append(p32s)
            p16_s.append(p16s)
            x_s.append(xset)

        PREFETCH = 8
        xt_pre = [emit_xprep(mt) for mt in range(PREFETCH)]

        for i in range(NEWTON_F16):
            do_sym = i in SYM_ITERS
            for s in range(NSETS):
                g1 = ps_g.tile([P, SETB, P], F32, tag="g")
                for j in range(SETB):
                    nc.tensor.matmul(g1[:, j, :], lhsT=p16_s[s][:, j, :],
                                     rhs=x_s[s][:, j, :], start=True, stop=True)
                t1n = nrot.tile([P, SETB, P], F16, tag="t1n")
                nc.scalar.activation(t1n[:], g1[:],      # -T1, off the DVE
                                     mybir.ActivationFunctionType.Copy,
                                     scale=-1.0)
                g2 = ps_g.tile([P, SETB, P], F32, tag="g")
                for j in range(SETB):                    # X' = X(2I) - X T1
                    nc.tensor.matmul(g2[:, j, :], lhsT=x_s[s][:, j, :],
                                     rhs=two_eye16[:], start=True, stop=False)
                    nc.tensor.matmul(g2[:, j, :], lhsT=x_s[s][:, j, :],
                                     rhs=t1n[:, j, :], start=False, stop=True)
                xset = nxp.tile([P, SETB, P], F16, tag=f"x{s}",
                                name=f"x{s}_{i}")
                if not do_sym:
                    acp(xset[:], g2[:])
                else:
                    xc = nrot.tile([P, SETB, P], F32, tag="xc")
                    cp(xc[:], g2[:])
                    tpg = ps_tp.tile([P, SETB, P], F32, tag="tp")
                    for j in range(SETB):
                        nc.tensor.transpose(tpg[:, j, :], xc[:, j, :],
                                            ident32[:])
                    add(xc[:], xc[:], tpg[:])
                    nc.scalar.activation(xset[:], xc[:],
                                         mybir.ActivationFunctionType.Copy,
                                         scale=0.5)
                x_s[s] = xset

        xf_s = []
        for s in range(NSETS):
            xf = nxp.tile([P, SETB, P], F32, tag=f"xf{s}", name=f"xf{s}_init")
            acp(xf[:], x_s[s][:])
            xf_s.append(xf)
        for i in range(NEWTON_F32):
            for s in range(NSETS):
                g1 = ps_g.tile([P, SETB, P], F32, tag="g")
                for j in range(SETB):
                    nc.tensor.matmul(g1[:, j, :], lhsT=p32_s[s][:, j, :],
                                     rhs=xf_s[s][:, j, :], start=True,
                                     stop=True)
                uf = nrot.tile([P, SETB, P], F32, tag="uf")
                sub(uf[:], bc(twoI), g1[:])
                g2 = ps_g.tile([P, SETB, P], F32, tag="g")
                for j in range(SETB):
                    nc.tensor.matmul(g2[:, j, :], lhsT=xf_s[s][:, j, :],
                                     rhs=uf[:, j, :], start=True, stop=True)
                xf = nxp.tile([P, SETB, P], F32, tag=f"xf{s}",
                              name=f"xf{s}_{i}")
                acp(xf[:], g2[:])
                xf_s[s] = xf

        # Q = B @ X with B^T = (1+e)I + (2+e)S + S^2 = eI12 + (2+e)S - P
        q_s = []
        for s in range(NSETS):
            bt = nrot.tile([P, SETB, P], F32, tag="bt")
            nc.vector.tensor_scalar(bt[:], s_s[s][:], float(2.0 + EPS), None,
                                    mybir.AluOpType.mult)
            add(bt[:], bt[:], bc(eI12))
            sub(bt[:], bt[:], p32_s[s][:])
            g = ps_g.tile([P, SETB, P], F32, tag="g")
            for j in range(SETB):
                nc.tensor.matmul(g[:, j, :], lhsT=bt[:, j, :],
                                 rhs=xf_s[s][:, j, :], start=True, stop=True)
            qset = qpool.tile([P, SETB, P], MAIN_DT, tag=f"q{s}", name=f"q{s}")
            acp(qset[:], g[:])
            q_s.append(qset)

        def q_lhsT(n):
            return q_s[n // SETB][:, n % SETB, :]

        # ---------- Phase B: W^T (streamed in 512-col groups) then filtT ----------
        for og in range(O_TILES):          # six 512-wide output column groups
            part = og // 2                 # which of q/k/v this group belongs to
            wts = wtsp.tile([P, NBLK, 512], MAIN_DT, tag="wts")
            for j4 in range(4):            # four 128-row W tiles per group
                ot = og * 4 + j4
                wrow = wpool.tile([P, HIDDEN], F32, tag="wrow")
                nc.sync.dma_start(wrow[:], w[ts(ot, P), :])
                for kh in range(2):                     # two 4-block groups
                    tpg = ps_tp.tile([P, SETB, P], F32, tag="tp")
                    for k4 in range(SETB):
                        k = kh * SETB + k4
                        nc.tensor.transpose(tpg[:, k4, :], wrow[:, ts(k, P)],
                                            ident32[:])
                    acp(wts[:, ts(kh, SETB), ts(j4, P)], tpg[:])
            for k in range(NBLK):
                fg = ps_g.tile([P, 512], F32, tag="g")
                nc.tensor.matmul(fg[:], lhsT=q_lhsT(part * NBLK + k),
                                 rhs=wts[:, k, :], start=True, stop=True)
                acp(ft[k][og][:], fg[:])

        _CACHE["xt_pre"] = xt_pre

    # ---------- Phase C: main matmul ----------
    xt_pre = _CACHE.pop("xt_pre")
    with tc.tile_pool(name="ps_out", bufs=6, space="PSUM") as ps_out:
        for mt in range(M_TILES):
            xt = xt_pre[mt] if mt < len(xt_pre) else emit_xprep(mt)
            psums = [ps_out.tile([P, 512], F32, tag="po", name=f"po{mt}_{i}")
                     for i in range(O_TILES)]
            for k in range(NBLK):
                for o in range(O_TILES):
                    nc.tensor.matmul(psums[o][:], lhsT=xt[:, k, :],
                                     rhs=ft[k][o][:],
                                     start=(k == 0), stop=(k == NBLK - 1))
            for o in range(O_TILES):
                ob = obp.tile([P, 512], F32, tag="ob", name=f"ob{mt}_{o}")
                add(ob[:], psums[o][:], bias_bc[:, ts(o, 512)])
                nc.sync.dma_start(out[ts(mt, P), ts(o, 512)], ob[:])


_CACHE = {}


def build():
    if "nc" in _CACHE:
        return _CACHE["nc"]
    import contextlib

    nc = bacc.Bacc("TRN2", target_bir_lowering=False, debug=False)
    with tile.TileContext(nc) as tc:
        with contextlib.ExitStack() as ctx:
            build_body(ctx, tc)
    nc.compile()
    _CACHE["nc"] = nc
    return nc


def make_in_maps(attn_weight, bias, x, q_R, k_R, v_R):
    rmat = np.ascontiguousarray(
        np.concatenate([q_R, k_R, v_R], axis=0), dtype=np.float32)
    w = np.ascontiguousarray(attn_weight, dtype=np.float32)
    b = np.ascontiguousarray(bias, dtype=np.float32)
    return [
        {"x": np.ascontiguousarray(x[c], dtype=np.float32),
         "w": w, "bias": b, "rmat": rmat}
        for c in range(N_CORES)
    ]


def kernel(attn_weight, bias, x, q_R, k_R, v_R, **run_kwargs):
    nc = build()
    in_maps = make_in_maps(attn_weight, bias, x, q_R, k_R, v_R)
    res = run_bass_kernel_spmd(nc, in_maps, core_ids=list(range(N_CORES)),
                               **run_kwargs)
    out = np.stack([res.results[c]["out"] for c in range(N_CORES)], axis=0)
    _CACHE["last_results"] = res
    return out
